# revision 10
# baseline (speedup 1.0000x reference)
"""DCN kernel for 8 trn2 NeuronCores (Bass/Tile).

Math: the deep stack (BN -> Linear x3 -> BN each) has NO nonlinearity in
eval mode, so it collapses to a weight-side matvec chain; the DCN cross
layers are rank-1 and collapse to per-row scalar recurrences over a handful
of dot products of x0 with fixed vectors.

Per core (512 batch rows):
  - dma_gather embedding rows (26 tables packed 3-per-gather, int16 idxs)
  - PE-transpose x0 tiles -> D = x0 @ U  (U = [cw0,cw1,cw2,px,vx]) in PSUM
  - weight chain: q3=a3*ph; r2=W3.T q3; q2=a2*r2; r1=W2.T q2; q1=a1*r1;
    r0=W1.T q1; q0=a0*r0 -> U col 4; scalar constants k_i summed on-device
  - cross recurrence on DVE, sigmoid on ACT, DMA out [128,4]
"""

import sys

if "/opt/trn_rl_repo" not in sys.path:
    sys.path.insert(0, "/opt/trn_rl_repo")

import numpy as np

import concourse.bacc as bacc
import concourse.mybir as mybir
import concourse.tile as tile
from concourse.bass_utils import run_bass_kernel_spmd

F32 = mybir.dt.float32
I16 = mybir.dt.int16
AF = mybir.ActivationFunctionType
OP = mybir.AluOpType
AX = mybir.AxisListType

B, F, V, D = 4096, 26, 10000, 64
NCORES = 8
BC = B // NCORES          # 512 rows per core
D0 = F * D                # 1664
KCH = 13                  # 128-wide feature chunks in D0
CCH = 4                   # 128-wide batch chunks per core
EPS = 1e-5
NG = 13                   # gather groups (2 tables each -> contiguous tiles)
GSLOT = 64                # idx free-dim slots per group (1024/16)

_CACHED = None
_LAST_RES = None


def _build():
    nc = bacc.Bacc("TRN2", target_bir_lowering=False)

    emb = nc.dram_tensor("emb", [F * V, D], F32, kind="ExternalInput")
    gidx = nc.dram_tensor("gidx", [128, NG * GSLOT], I16, kind="ExternalInput")
    numb_p = nc.dram_tensor("numb_p", [128, CCH * 13], F32, kind="ExternalInput")
    w1 = nc.dram_tensor("w1", [1024, 1677], F32, kind="ExternalInput")
    w2 = nc.dram_tensor("w2", [512, 1024], F32, kind="ExternalInput")
    w3 = nc.dram_tensor("w3", [256, 512], F32, kind="ExternalInput")
    bn0 = nc.dram_tensor("bn0", [4, 1677], F32, kind="ExternalInput")
    bn1 = nc.dram_tensor("bn1", [4, 1024], F32, kind="ExternalInput")
    bn2 = nc.dram_tensor("bn2", [4, 512], F32, kind="ExternalInput")
    bn3 = nc.dram_tensor("bn3", [4, 256], F32, kind="ExternalInput")
    b1 = nc.dram_tensor("b1", [1024], F32, kind="ExternalInput")
    b2 = nc.dram_tensor("b2", [512], F32, kind="ExternalInput")
    b3 = nc.dram_tensor("b3", [256], F32, kind="ExternalInput")
    cross_w = nc.dram_tensor("cross_w", [3, D0], F32, kind="ExternalInput")
    cross_b = nc.dram_tensor("cross_b", [3], F32, kind="ExternalInput")
    pred_w = nc.dram_tensor("pred_w", [1, 1920], F32, kind="ExternalInput")
    pred_b = nc.dram_tensor("pred_b", [1], F32, kind="ExternalInput")
    iden = nc.dram_tensor("iden", [128, 128], F32, kind="ExternalInput")
    outp = nc.dram_tensor("outp", [128, CCH], F32, kind="ExternalOutput")
    dbg = nc.dram_tensor("dbg", [128, 64], F32, kind="ExternalOutput")
    dbgx = nc.dram_tensor("dbgx", [128, 1024], F32, kind="ExternalOutput")

    with tile.TileContext(nc) as tc:
        with (
            tc.tile_pool(name="w1p", bufs=1) as w1p,
            tc.tile_pool(name="big", bufs=1) as big,
            tc.tile_pool(name="sm", bufs=1) as smp,
            tc.tile_pool(name="ts", bufs=1) as tsp,
            tc.tile_pool(name="scr", bufs=4) as scr,
            tc.tile_pool(name="ps_tp", bufs=2, space="PSUM") as ps_tp,
            tc.tile_pool(name="ps_ch", bufs=2, space="PSUM") as ps_ch,
            tc.tile_pool(name="ps_d", bufs=2, space="PSUM") as ps_d,
            tc.tile_pool(name="ps_m", bufs=1, space="PSUM") as ps_m,
        ):
            # ---------------- persistent SBUF tiles ----------------
            W1s = w1p.tile([128, 8, 1677], F32)
            W2s = big.tile([128, 4, 1024], F32)
            W3s = big.tile([128, 2, 512], F32)
            x0s = big.tile([128, F * CCH * D], F32)          # [p, f, c, d] flat
            gix = smp.tile([128, NG * GSLOT], I16)
            idn = smp.tile([128, 128], F32)
            nb = smp.tile([128, CCH, 13], F32)
            stage = smp.tile([128, 128], F32)                # small-vec rows
            stg2 = smp.tile([64, 128], F32)                  # cw0..2,px rows
            cols = smp.tile([128, 128], F32)                 # transposed smalls
            u4s = smp.tile([128, 52], F32)
            um = smp.tile([128, KCH, 5], F32)                # U col layout
            unm = smp.tile([16, 5], F32)                     # numb rows of U
            a0t = smp.tile([128, 14], F32)
            c0t = smp.tile([128, 14], F32)
            a1t = smp.tile([128, 8], F32)
            c1t = smp.tile([128, 8], F32)
            a2t = smp.tile([128, 4], F32)
            c2t = smp.tile([128, 4], F32)
            a3t = smp.tile([128, 2], F32)
            c3t = smp.tile([128, 2], F32)
            q3t = smp.tile([128, 2], F32)
            q2t = smp.tile([128, 4], F32)
            q1t = smp.tile([128, 8], F32)
            racc2 = smp.tile([128, 4], F32)
            racc1 = smp.tile([128, 8], F32)
            racc0 = smp.tile([128, 14], F32)
            kacc = smp.tile([128, 1], F32)
            ones1 = smp.tile([1, 128], F32)
            onesC = smp.tile([128, 1], F32)
            cb3 = smp.tile([1, 3], F32)
            pbs = smp.tile([1, 1], F32)
            grow = smp.tile([1, 8], F32)
            gbs = smp.tile([128, 8], F32)
            ds = smp.tile([128, CCH, 5], F32)
            osb = smp.tile([128, CCH], F32)
            # x0T tiles: 13 quads of [128, 512] per c is too much; keep
            # per-(c,k) [128,128] packed 4-wide for cheap copies.
            Tq = [[tsp.tile([128, 512], F32, tag=f"tq{c}_{q}", name=f"tq{c}_{q}") for q in range(4)]
                  for c in range(CCH)]
            nTs = tsp.tile([16, CCH * 128], F32)

            # ---------------- DMAs ----------------
            nc.sync.dma_start(gix[:, :], gidx[:, :])
            nc.sync.dma_start(idn[:, :], iden[:, :])
            nc.sync.dma_start(nb[:, :, :], numb_p[:, :].rearrange(
                "p (c j) -> p c j", c=CCH))
            nc.vector.memset(stage[:, :], 1.0)
            # bn rows -> stage rows (column layouts appear after transpose)
            offs_bn = {"bn0": (bn0, 1677, 14, 0), "bn1": (bn1, 1024, 8, 56),
                       "bn2": (bn2, 512, 4, 88), "bn3": (bn3, 256, 2, 104)}
            for _, (t, dim, nk, base) in offs_bn.items():
                full = dim // 128
                for r in range(4):
                    p0 = base + r * nk
                    nc.scalar.dma_start(
                        stage[p0:p0 + full, :],
                        t[r, 0:full * 128].rearrange("(c e) -> c e", e=128))
                    if dim % 128:
                        rem = dim % 128
                        nc.scalar.dma_start(
                            stage[p0 + full:p0 + full + 1, 0:rem],
                            t[r, full * 128:dim].rearrange("(c e) -> c e", e=rem))
            nc.scalar.dma_start(stage[112:120, :],
                                b1[0:1024].rearrange("(c e) -> c e", e=128))
            nc.scalar.dma_start(stage[120:124, :],
                                b2[0:512].rearrange("(c e) -> c e", e=128))
            nc.scalar.dma_start(stage[124:126, :],
                                b3[0:256].rearrange("(c e) -> c e", e=128))
            nc.scalar.dma_start(stage[126:128, :],
                                pred_w[0, 1664:1920].rearrange("(c e) -> c e", e=128))
            for j in range(3):
                nc.scalar.dma_start(
                    stg2[j * 13:(j + 1) * 13, :],
                    cross_w[j, :].rearrange("(c e) -> c e", e=128))
            nc.scalar.dma_start(stg2[39:52, :],
                                pred_w[0, 0:1664].rearrange("(c e) -> c e", e=128))
            nc.scalar.dma_start(cb3[0:1, :], cross_b[None, :])
            nc.scalar.dma_start(pbs[0:1, :], pred_b[None, :])
            # weights
            for kc in range(2):
                nc.sync.dma_start(W3s[:, kc, :], w3[kc * 128:(kc + 1) * 128, :])
            for kc in range(4):
                nc.sync.dma_start(W2s[:, kc, :], w2[kc * 128:(kc + 1) * 128, :])
            # gathers (gpsimd SWDGE). group k covers tables 2k,2k+1;
            # idx order i=(c*2+t)*128+p so x0s gets [p, k, c, f=t*64+d],
            # making every [128b,128f] transpose tile contiguous.
            for g in range(NG):
                nidx = 2 * BC
                out_ap = x0s[:, g * 512:(g + 1) * 512].rearrange(
                    "p (r d) -> p r d", d=D)
                nc.gpsimd.dma_gather(
                    out_ap,
                    emb[g * 2 * V:(g + 1) * 2 * V, :],
                    gix[:, g * GSLOT:(g + 1) * GSLOT],
                    nidx, nidx, D)
            for kc in range(8):
                nc.sync.dma_start(W1s[:, kc, :], w1[kc * 128:(kc + 1) * 128, :])

            # ---------------- small-vector transposes ----------------
            tps = ps_tp.tile([128, 512], F32, tag="tp")
            nc.tensor.transpose(tps[:, 0:128], stage[:, :], idn[:, :])
            nc.vector.tensor_copy(cols[:, :], tps[:, 0:128])
            tps2 = ps_tp.tile([128, 512], F32, tag="tp")
            nc.tensor.transpose(tps2[:, 0:52], stg2[0:52, :], idn[0:52, 0:52])
            nc.vector.tensor_copy(u4s[:, :], tps2[:, 0:52])
            # um cols 0..3  (um[p,k,j] = colsU[p, j*13+k])
            nc.vector.tensor_copy(
                um[:, :, 0:4],
                u4s[:, :].rearrange("p (j k) -> p k j", j=4))

            # bn affine params in column layout: a = g*rsqrt(v+eps), c = b - m*a
            for (aT, cT, nk, base) in ((a0t, c0t, 14, 0), (a1t, c1t, 8, 56),
                                       (a2t, c2t, 4, 88), (a3t, c3t, 2, 104)):
                g_ = cols[:, base:base + nk]
                b_ = cols[:, base + nk:base + 2 * nk]
                m_ = cols[:, base + 2 * nk:base + 3 * nk]
                v_ = cols[:, base + 3 * nk:base + 4 * nk]
                t = scr.tile([128, 14], F32, tag="sc")
                nc.vector.tensor_scalar(t[:, 0:nk], v_, EPS, None, OP.add)
                nc.scalar.activation(t[:, 0:nk], t[:, 0:nk], AF.Sqrt)
                nc.vector.reciprocal(t[:, 0:nk], t[:, 0:nk])
                nc.vector.tensor_mul(aT[:, :], g_, t[:, 0:nk])
                t2 = scr.tile([128, 14], F32, tag="sc")
                nc.vector.tensor_mul(t2[:, 0:nk], m_, aT[:, :])
                nc.vector.tensor_sub(cT[:, :], b_, t2[:, 0:nk])

            phc = cols[:, 126:128]
            b1c, b2c, b3c = cols[:, 112:120], cols[:, 120:124], cols[:, 124:126]

            def kpart(dst_init, bvec, qv, cvec, rv, nk):
                """kacc (+)= sum_free(bvec*qv + cvec*rv)"""
                t = scr.tile([128, 14], F32, tag="sc")
                nc.vector.tensor_mul(t[:, 0:nk], bvec, qv)
                t2 = scr.tile([128, 14], F32, tag="sc")
                nc.vector.tensor_mul(t2[:, 0:nk], cvec, rv)
                nc.vector.tensor_add(t[:, 0:nk], t[:, 0:nk], t2[:, 0:nk])
                red = scr.tile([128, 1], F32, tag="red")
                nc.vector.tensor_reduce(red[:, :], t[:, 0:nk], AX.X, OP.add)
                if dst_init:
                    nc.vector.tensor_copy(kacc[:, :], red[:, :])
                else:
                    nc.vector.tensor_add(kacc[:, :], kacc[:, :], red[:, :])

            # ---------------- weight chain ----------------
            nc.vector.tensor_mul(q3t[:, :], a3t[:, :], phc)
            kpart(True, b3c, q3t[:, :], c3t[:, :], phc, 2)

            # matvec layers: per-kc single-matmul groups into a fresh psum
            # bank, accumulate across kc on DVE (PSUM start=True clears
            # has_written for the WHOLE bank, so cross-kc psum accumulation
            # with per-column groups is unsafe).
            def matvec(Ws, qv, acc, nkc, nm, mtail=128):
                nc.vector.memset(acc[:, :], 0.0)
                for kc in range(nkc):
                    pt = ps_ch.tile([128, 14], F32, tag="ch", name=f"pt{nkc}_{kc}")
                    nc.vector.memset(pt[:, 0:nm], 0.0)
                    for m in range(nm):
                        mw = 128 if m < nm - 1 else mtail
                        nc.tensor.matmul(pt[0:mw, m:m + 1],
                                         Ws[:, kc, m * 128:m * 128 + mw],
                                         qv[:, kc:kc + 1],
                                         start=True, stop=True)
                    nc.vector.tensor_add(acc[:, 0:nm], acc[:, 0:nm],
                                         pt[:, 0:nm])

            matvec(W3s, q3t, racc2, 2, 4)
            nc.vector.tensor_mul(q2t[:, :], a2t[:, :], racc2[:, :])
            kpart(False, b2c, q2t[:, :], c2t[:, :], racc2[:, :], 4)

            matvec(W2s, q2t, racc1, 4, 8)
            nc.vector.tensor_mul(q1t[:, :], a1t[:, :], racc1[:, :])
            kpart(False, b1c, q1t[:, :], c1t[:, :], racc1[:, :], 8)

            matvec(W1s, q1t, racc0, 8, 14, mtail=13)
            # q0 -> U col 4 (x-part) and unm col 4 (numb part)
            nc.vector.tensor_mul(
                um[:, :, 4:5].rearrange("p k j -> p (k j)"),
                a0t[:, 0:13], racc0[:, 0:13])
            nc.vector.memset(unm[:, :], 0.0)
            nc.vector.tensor_mul(unm[0:13, 4:5], a0t[0:13, 13:14],
                                 racc0[0:13, 13:14])
            # k0 = c0 . r0  (split: full 13 cols + 13-row tail col)
            t = scr.tile([128, 14], F32, tag="sc")
            nc.vector.tensor_mul(t[:, 0:13], c0t[:, 0:13], racc0[:, 0:13])
            red = scr.tile([128, 1], F32, tag="red")
            nc.vector.tensor_reduce(red[:, :], t[:, 0:13], AX.X, OP.add)
            nc.vector.tensor_add(kacc[:, :], kacc[:, :], red[:, :])
            t13 = scr.tile([16, 1], F32, tag="t13")
            nc.vector.tensor_mul(t13[0:13, :], c0t[0:13, 13:14],
                                 racc0[0:13, 13:14])
            nc.vector.tensor_add(kacc[0:13, :], kacc[0:13, :], t13[0:13, :])

            # ---------------- S sums + G row + broadcast ----------------
            nc.vector.memset(ones1[:, :], 1.0)
            nc.vector.memset(onesC[:, :], 1.0)
            ured = scr.tile([128, 3], F32, tag="ured")
            nc.vector.tensor_reduce(
                ured[:, :], um[:, :, 1:4].rearrange("p k j -> p j k"),
                AX.X, OP.add)
            psm = ps_m.tile([128, 24], F32, tag="m")
            srow = psm[0:1, 0:8]
            nc.tensor.matmul(psm[0:1, 0:3], onesC[:, :], ured[:, :],
                             start=True, stop=True)
            kred = psm[0:1, 8:16]
            nc.tensor.matmul(psm[0:1, 8:9], onesC[:, :], kacc[:, :],
                             start=True, stop=True)
            # grow = [cb0*S1, cb0*S2, cb0*Sp, cb1*S2, cb1*Sp, cb2*Sp, K', 0]
            nc.vector.tensor_scalar(grow[0:1, 0:3], psm[0:1, 0:3],
                                    cb3[0:1, 0:1], None, OP.mult)
            nc.vector.tensor_scalar(grow[0:1, 3:5], psm[0:1, 1:3],
                                    cb3[0:1, 1:2], None, OP.mult)
            nc.vector.tensor_scalar(grow[0:1, 5:6], psm[0:1, 2:3],
                                    cb3[0:1, 2:3], None, OP.mult)
            nc.vector.tensor_scalar(grow[0:1, 6:7], psm[0:1, 8:9],
                                    pbs[0:1, 0:1], None, OP.add)
            nc.vector.memset(grow[0:1, 7:8], 0.0)
            gbp = psm[:, 16:24]
            nc.tensor.matmul(gbp, ones1[:, :], grow[0:1, :],
                             start=True, stop=True)
            nc.vector.tensor_copy(gbs[:, :], gbp)

            # ---------------- x0 transposes + D matmuls ----------------
            for c in range(CCH):
                for q in range(4):
                    ks = range(4 * q, min(4 * q + 4, KCH))
                    tp = ps_tp.tile([128, 512], F32, tag="tp")
                    for i, k in enumerate(ks):
                        off = (k * CCH + c) * 128
                        nc.tensor.transpose(
                            tp[:, i * 128:(i + 1) * 128],
                            x0s[:, off:off + 128], idn[:, :])
                    w = len(ks) * 128
                    eng = nc.vector if (q % 2 == 0) else nc.scalar
                    if eng is nc.vector:
                        nc.vector.tensor_copy(Tq[c][q][:, 0:w], tp[:, 0:w])
                    else:
                        nc.scalar.activation(Tq[c][q][:, 0:w], tp[:, 0:w],
                                             AF.Copy)
                ntp = ps_tp.tile([128, 512], F32, tag="tp")
                nc.tensor.transpose(ntp[0:13, 0:128], nb[:, c, :], idn[:, :])
                nc.vector.tensor_copy(nTs[0:13, c * 128:(c + 1) * 128],
                                      ntp[0:13, 0:128])

            for c in range(CCH):
                dp = ps_d.tile([128, 5], F32, tag="d")
                for k in range(KCH):
                    nc.tensor.matmul(dp[:, :],
                                     Tq[c][k // 4][:, (k % 4) * 128:(k % 4 + 1) * 128],
                                     um[:, k, :],
                                     start=(k == 0), stop=False)
                nc.tensor.matmul(dp[:, :], nTs[0:13, c * 128:(c + 1) * 128],
                                 unm[0:13, :], start=False, stop=True)
                nc.vector.tensor_copy(ds[:, c, :], dp[:, :])

            # ---------------- cross recurrence + sigmoid ----------------
            dcol = lambda j: ds[:, :, j:j + 1].rearrange("p c j -> p (c j)")
            gcol = lambda j: gbs[:, j:j + 1]
            s0p1 = scr.tile([128, CCH], F32, tag="rc")
            nc.vector.tensor_scalar(s0p1[:, :], dcol(0), 1.0, None, OP.add)
            A1 = scr.tile([128, CCH], F32, tag="rc")
            nc.vector.tensor_mul(A1[:, :], dcol(1), s0p1[:, :])
            nc.vector.tensor_scalar(A1[:, :], A1[:, :], gcol(0), 1.0, OP.add,
                                    OP.add)
            A2 = scr.tile([128, CCH], F32, tag="rc")
            nc.vector.tensor_mul(A2[:, :], dcol(2), s0p1[:, :])
            nc.vector.tensor_scalar(A2[:, :], A2[:, :], gcol(1), None, OP.add)
            P = scr.tile([128, CCH], F32, tag="rc")
            nc.vector.tensor_mul(P[:, :], dcol(3), s0p1[:, :])
            nc.vector.tensor_scalar(P[:, :], P[:, :], gcol(2), None, OP.add)
            nc.vector.tensor_mul(A2[:, :], A2[:, :], A1[:, :])
            nc.vector.tensor_scalar(A2[:, :], A2[:, :], gcol(3), 1.0, OP.add,
                                    OP.add)
            nc.vector.tensor_mul(P[:, :], P[:, :], A1[:, :])
            nc.vector.tensor_scalar(P[:, :], P[:, :], gcol(4), None, OP.add)
            nc.vector.tensor_mul(P[:, :], P[:, :], A2[:, :])
            nc.vector.tensor_scalar(P[:, :], P[:, :], gcol(5), None, OP.add)
            nc.vector.tensor_add(P[:, :], P[:, :], dcol(4))
            nc.vector.tensor_scalar(P[:, :], P[:, :], gcol(6), None, OP.add)
            nc.scalar.activation(osb[:, :], P[:, :], AF.Sigmoid)
            nc.sync.dma_start(outp[:, :], osb[:, :])
            # ---- debug dumps ----
            dbt = smp.tile([128, 64], F32)
            nc.vector.tensor_copy(dbt[:, 0:20], ds[:, :, :].rearrange("p c j -> p (c j)"))
            nc.vector.tensor_copy(dbt[:, 20:28], gbs[:, :])
            nc.vector.tensor_copy(dbt[:, 28:29], kacc[:, :])
            nc.vector.tensor_copy(dbt[:, 29:42], um[:, :, 4:5].rearrange("p k j -> p (k j)"))
            nc.vector.tensor_copy(dbt[:, 42:50], q1t[:, :])
            nc.vector.tensor_copy(dbt[:, 50:54], q2t[:, :])
            nc.vector.tensor_copy(dbt[:, 54:56], q3t[:, :])
            nc.vector.tensor_copy(dbt[:, 56:60], um[:, 0, 0:4])
            nc.vector.tensor_copy(dbt[:, 60:62], a0t[:, 0:2])
            nc.sync.dma_start(dbg[:, :], dbt[:, :])
            dxt = smp.tile([128, 1024], F32)
            nc.vector.tensor_copy(dxt[:, 0:512], x0s[:, 0:512])
            nc.vector.tensor_copy(dxt[:, 512:640], Tq[0][0][:, 0:128])
            nc.vector.tensor_copy(dxt[:, 640:768], nTs[0:13, 0:128].partition_broadcast(128) if False else Tq[0][0][:, 128:256])
            nc.vector.tensor_copy(dxt[:, 768:1024], u4s[:, 0:52].partition_broadcast(128) if False else Tq[0][0][:, 256:512])
            nc.sync.dma_start(dbgx[:, :], dxt[:, :])

    nc.compile()
    return nc


def _prep_core(cat_c, numb_c):
    """Per-core host layout prep: int16 gather idxs + numb permute."""
    gidx = np.zeros((128, NG * GSLOT), np.int16)
    for g in range(NG):
        v = cat_c[:, 2 * g:2 * g + 2].astype(np.int32)              # [512, 2]
        v = v + (np.arange(2, dtype=np.int32) * V)[None, :]
        # rank r = c*2 + t ; i = r*128 + p ; b = c*128 + p
        flat = v.reshape(CCH, 128, 2).transpose(0, 2, 1).reshape(2 * BC)
        w = flat.reshape(-1, 16).T.astype(np.int16)                 # [16, 64]
        gidx[:, g * GSLOT:(g + 1) * GSLOT] = np.tile(w, (8, 1))
    nbp = np.ascontiguousarray(
        numb_c.reshape(CCH, 128, 13).transpose(1, 0, 2)).reshape(128, CCH * 13)
    return gidx, nbp


def kernel(**inputs):
    global _CACHED
    if _CACHED is None:
        _CACHED = _build()
    nc = _CACHED

    f32 = lambda k: np.ascontiguousarray(np.asarray(inputs[k], np.float32))
    cat = np.asarray(inputs["cat_features"])
    shared = {
        "emb": f32("emb_tables").reshape(F * V, D),
        "w1": f32("w1"), "w2": f32("w2"), "w3": f32("w3"),
        "bn0": f32("bn0"), "bn1": f32("bn1"), "bn2": f32("bn2"),
        "bn3": f32("bn3"),
        "b1": f32("b1"), "b2": f32("b2"), "b3": f32("b3"),
        "cross_w": f32("cross_w"), "cross_b": f32("cross_b"),
        "pred_w": f32("pred_w"), "pred_b": f32("pred_b"),
        "iden": np.eye(128, dtype=np.float32),
    }
    numb = f32("numb_features")
    in_maps = []
    for i in range(NCORES):
        gidx, nbp = _prep_core(cat[i * BC:(i + 1) * BC],
                               numb[i * BC:(i + 1) * BC])
        in_maps.append({**shared, "gidx": gidx, "numb_p": nbp})

    res = run_bass_kernel_spmd(nc, in_maps, list(range(NCORES)))
    global _LAST_RES
    _LAST_RES = res
    out = np.empty((B, 1), np.float32)
    for i in range(NCORES):
        out[i * BC:(i + 1) * BC, 0] = res.results[i]["outp"].T.reshape(BC)
    return out
